# revision 4
# baseline (speedup 1.0000x reference)
"""Distributed TRN2 Bass kernel for nn_Attention_2207613190299.

Sharding: batch n=2 x 4 column-shards of the 4096-wide correlation matrix
-> 8 cores. Each core computes, for its batch b and column range
[q0, q0+1024):
  - fa/fb 1x1 conv (256->64) + instance norm + leaky relu + spatial mean
    subtraction (on device)
  - energy^T columns via float32r matmul, softmax over p with the exp scale
    100/(||fb_p||+eps) folded into the activation (energy <= 100 bound used
    instead of a max pass)
  - softmax denominators S[q] and fc-warp numerators via a ones-augmented
    bf16 matmul over the exp tiles
  - corr[:, q0:q0+1024] normalized and DMA'd out
The tiny fc conv path (two 3x3 stride-2 convs on [2,3,256,256]) and the
final fc_warp upsample chain run on host numpy (~0.3% of FLOPs).
"""
import os
import sys
import types

import numpy as np
import ml_dtypes


def _install_prof_shim():
    """Let run_bass_kernel_spmd(trace=True) NTFF-profile under axon even
    though the agent image's antenv lacks axon_hooks."""
    if "antenv.axon_hooks" in sys.modules:
        return
    try:
        import antenv
    except ImportError:
        return
    mod = types.ModuleType("antenv.axon_hooks")
    _hook = [None]
    mod.set_axon_ntff_profile_hook = lambda h: _hook.__setitem__(0, h)
    mod.get_axon_ntff_profile_hook = lambda: _hook[0]
    sys.modules["antenv.axon_hooks"] = mod
    antenv.axon_hooks = mod
    try:
        from trn_agent_boot.trn_boot import _ntff_profile_via_ctypes

        mod.set_axon_ntff_profile_hook(
            _ntff_profile_via_ctypes("/opt/axon/libaxon_pjrt.so")
        )
    except Exception:
        pass


_install_prof_shim()

import concourse.bass as bass
import concourse.mybir as mybir
import concourse.tile as tile
from concourse import bacc
from concourse.bass import ts, ds
from concourse.bass_utils import run_bass_kernel_spmd

EPS = 1e-5
ALPHA = 100.0
N_CORES = 8
C_IN = 256
C_MID = 64
HW = 4096  # 64*64
QS = HW // 4  # 1024 columns per core
NPT = HW // 128  # 32 p-tiles

LAST_RESULT = {}

_dt = mybir.dt
_f32 = _dt.float32
_f32r = _dt.float32r
_bf16 = _dt.bfloat16

_COMPILED = {}


def _build():
    nc = bacc.Bacc("TRN2", target_bir_lowering=False, debug=False,
                   num_devices=N_CORES)

    fa_d = nc.dram_tensor("fa_raw", [C_IN, HW], _f32, kind="ExternalInput").ap()
    fb_d = nc.dram_tensor("fb_raw", [C_IN, HW], _f32, kind="ExternalInput").ap()
    wa_d = nc.dram_tensor("waT", [C_IN, C_MID], _f32, kind="ExternalInput").ap()
    wb_d = nc.dram_tensor("wbT", [C_IN, C_MID], _f32, kind="ExternalInput").ap()
    fct_d = nc.dram_tensor("fct", [128, NPT, 4], _bf16, kind="ExternalInput").ap()

    corr_d = nc.dram_tensor("corr", [HW, QS], _f32, kind="ExternalOutput").ap()
    numS_d = nc.dram_tensor("numS", [4, QS], _f32, kind="ExternalOutput").ap()

    with tile.TileContext(nc) as tc:
        with (
            tc.tile_pool(name="raw", bufs=2) as p_raw,
            tc.tile_pool(name="work", bufs=2) as p_work,
            tc.tile_pool(name="keep", bufs=1) as p_keep,
            tc.tile_pool(name="small", bufs=1) as p_small,
            tc.tile_pool(name="stage", bufs=2) as p_stage,
            tc.tile_pool(name="loc", bufs=3) as p_loc,
            tc.tile_pool(name="epool", bufs=NPT) as p_e,
            tc.tile_pool(name="dram", bufs=1, space="DRAM") as p_dram,
        ):
            ones_f = p_small.tile([C_MID, 128], _f32)
            nc.vector.memset(ones_f[:], 1.0)
            ones_r = p_small.tile([C_MID, 128], _f32r)
            nc.vector.tensor_copy(ones_r[:], ones_f[:])
            bias_m100 = p_small.tile([128, 1], _f32)
            nc.vector.memset(bias_m100[:], -ALPHA)
            eps_t = p_small.tile([128, 1], _f32)
            nc.vector.memset(eps_t[:], EPS)

            fct_t = p_small.tile([128, NPT, 4], _bf16)
            nc.sync.dma_start(out=fct_t[:], in_=fct_d)

            waT_t = p_small.tile([128, 2, C_MID], _f32)
            nc.sync.dma_start(out=waT_t[:], in_=wa_d.rearrange("(k p) m -> p k m", p=128))
            wbT_t = p_small.tile([128, 2, C_MID], _f32)
            nc.sync.dma_start(out=wbT_t[:], in_=wb_d.rearrange("(k p) m -> p k m", p=128))

            # ---------------- fa / fb prep ----------------
            # returns (z_r tile or None, za_n) depending on side
            def prep(raw_d, wT_t, local_only, pname):
                with tc.tile_pool(name=pname, bufs=8, space="PSUM") as p_ps:
                    raw0 = p_raw.tile([128, HW], _f32, tag="raw")
                    raw1 = p_raw.tile([128, HW], _f32, tag="raw")
                    nc.sync.dma_start(out=raw0[:], in_=raw_d[0:128, :])
                    nc.sync.dma_start(out=raw1[:], in_=raw_d[128:256, :])

                    xc = p_work.tile([C_MID, HW], _f32, tag="w")
                    stats = p_small.tile([C_MID, 8, 6], _f32, tag="stats")
                    for ch in range(8):
                        pt = p_ps.tile([C_MID, 512], _f32, tag="pp")
                        nc.tensor.matmul(pt[:], wT_t[:, 0, :],
                                         raw0[:, ts(ch, 512)],
                                         start=True, stop=False)
                        nc.tensor.matmul(pt[:], wT_t[:, 1, :],
                                         raw1[:, ts(ch, 512)],
                                         start=False, stop=True)
                        nc.vector.bn_stats(out=stats[:, ch, :], in_=pt[:])
                        nc.scalar.copy(out=xc[:, ts(ch, 512)], in_=pt[:])

                    mv = p_small.tile([C_MID, 2], _f32, tag="mv")
                    nc.vector.bn_aggr(out=mv[:], in_=stats[:])
                    sd = p_small.tile([C_MID, 1], _f32, tag="sd")
                    nc.scalar.activation(out=sd[:], in_=mv[:, 1:2],
                                         func=mybir.ActivationFunctionType.Sqrt,
                                         bias=eps_t[0:C_MID], scale=1.0)
                    rstd = p_small.tile([C_MID, 1], _f32, tag="rstd")
                    nc.vector.reciprocal(out=rstd[:], in_=sd[:])
                    nbias = p_small.tile([C_MID, 1], _f32, tag="nbias")
                    # nbias = -mean * rstd
                    nc.vector.scalar_tensor_tensor(
                        out=nbias[:], in0=mv[:, 0:1], scalar=-1.0, in1=rstd[:],
                        op0=mybir.AluOpType.mult, op1=mybir.AluOpType.mult)

                    # t = rstd*x + nbias   (instance norm affine)
                    tn = p_work.tile([C_MID, HW], _f32, tag="w")
                    nc.scalar.activation(out=tn[:], in_=xc[:],
                                         func=mybir.ActivationFunctionType.Identity,
                                         bias=nbias[:], scale=rstd[:])
                    # y = max(t, 0.2t), ysum = sum(y) over hw
                    y = p_work.tile([C_MID, HW], _f32, tag="w")
                    ysum = p_small.tile([C_MID, 1], _f32, tag="ysum")
                    nc.vector.scalar_tensor_tensor(
                        out=y[:], in0=tn[:], scalar=0.2, in1=tn[:],
                        op0=mybir.AluOpType.mult, op1=mybir.AluOpType.max,
                        accum_out=ysum[:])
                    mean2 = p_small.tile([C_MID, 1], _f32, tag="mean2")
                    nc.vector.tensor_scalar_mul(mean2[:], ysum[:], 1.0 / HW)

                    if local_only:
                        # fa side: z, squares, norms only on local columns
                        za = p_loc.tile([C_MID, QS], _f32, tag="loc")
                        nc.vector.tensor_scalar(
                            out=za[:], in0=y[:, ds(Q0, QS)], scalar1=mean2[:],
                            scalar2=None, op0=mybir.AluOpType.subtract)
                        sq = p_loc.tile([C_MID, QS], _f32r, tag="loc")
                        nc.vector.tensor_mul(sq[:], za[:], za[:])
                        nrm = p_loc.tile([128, QS], _f32, tag="loc")
                        for ch in range(2):
                            pn = p_ps.tile([128, 512], _f32, tag="pp")
                            nc.tensor.matmul(pn[:], ones_r[:],
                                             sq[:, ts(ch, 512)],
                                             start=True, stop=True)
                            nc.scalar.activation(
                                out=nrm[:, ts(ch, 512)], in_=pn[:],
                                func=mybir.ActivationFunctionType.Sqrt)
                        inva = p_loc.tile([C_MID, QS], _f32, tag="loc")
                        nc.vector.tensor_scalar_add(inva[:], nrm[0:C_MID, :], EPS)
                        nc.vector.reciprocal(out=inva[:], in_=inva[:])
                        za_n = p_keep.tile([C_MID, QS], _f32r, tag="za_n")
                        nc.vector.tensor_mul(za_n[:], za[:], inva[:])
                        return za_n
                    else:
                        # fb side: full z (f32r) + transposed 100/(nrm+eps)
                        z_r = p_keep.tile([C_MID, HW], _f32r, tag="z_r")
                        nc.vector.tensor_scalar(
                            out=z_r[:], in0=y[:], scalar1=mean2[:],
                            scalar2=None, op0=mybir.AluOpType.subtract)
                        sq = p_work.tile([C_MID, HW], _f32r, tag="w")
                        nc.vector.tensor_mul(sq[:], z_r[:], z_r[:])
                        nrm = p_work.tile([128, HW], _f32, tag="w")
                        for ch in range(8):
                            pn = p_ps.tile([128, 512], _f32, tag="pp")
                            nc.tensor.matmul(pn[:], ones_r[:],
                                             sq[:, ts(ch, 512)],
                                             start=True, stop=True)
                            nc.scalar.activation(
                                out=nrm[:, ts(ch, 512)], in_=pn[:],
                                func=mybir.ActivationFunctionType.Sqrt)
                        # transpose nrm row0 [1, 4096] -> [128, 32] via DRAM
                        scratch = p_dram.tile([1, HW], _f32)
                        nc.sync.dma_start(out=scratch[:], in_=nrm[0:1, :])
                        nbT = p_small.tile([128, NPT], _f32, tag="nbT")
                        nc.sync.dma_start(
                            out=nbT[:],
                            in_=scratch[0, :].rearrange("(t r) -> r t", r=128))
                        nc.vector.tensor_scalar_add(nbT[:], nbT[:], EPS)
                        nc.vector.reciprocal(out=nbT[:], in_=nbT[:])
                        scaleT = p_small.tile([128, NPT], _f32, tag="scaleT")
                        nc.vector.tensor_scalar_mul(scaleT[:], nbT[:], ALPHA)
                        return z_r, scaleT

            za_n = prep(fa_d, waT_t, local_only=True, pname="ppa")
            zb_r, scaleT = prep(fb_d, wbT_t, local_only=False, pname="ppb")

            # ---------------- energy + exp + num/S ----------------
            with (
                tc.tile_pool(name="pe", bufs=2, space="PSUM") as p_pe,
                tc.tile_pool(name="pnum", bufs=1, space="PSUM") as p_pnum,
                tc.tile_pool(name="pmisc", bufs=1, space="PSUM") as p_pmisc,
            ):
                pnum = p_pnum.tile([4, QS], _f32)
                e_tiles = []
                for t in range(NPT):
                    pe = p_pe.tile([128, QS], _f32, tag="pe")
                    for ch in range(2):
                        nc.tensor.matmul(pe[:, ts(ch, 512)],
                                         zb_r[:, ts(t, 128)],
                                         za_n[:, ts(ch, 512)],
                                         start=True, stop=True)
                    et = p_e.tile([128, QS], _bf16, tag="E")
                    nc.scalar.activation(out=et[:], in_=pe[:],
                                         func=mybir.ActivationFunctionType.Exp,
                                         bias=bias_m100[:],
                                         scale=scaleT[:, ts(t, 1)])
                    for ch in range(2):
                        nc.tensor.matmul(pnum[:, ts(ch, 512)], fct_t[:, t, :],
                                         et[:, ts(ch, 512)],
                                         start=(t == 0), stop=(t == NPT - 1),
                                         skip_group_check=True)
                    e_tiles.append(et)

                # numS out
                numS_sb = p_small.tile([4, QS], _f32, tag="numS")
                nc.vector.tensor_copy(numS_sb[:], pnum[:])
                nc.sync.dma_start(out=numS_d, in_=numS_sb[:])

                # invS broadcast [128, QS]
                s_row = p_small.tile([1, QS], _f32r, tag="s_row")
                nc.vector.tensor_copy(s_row[:], pnum[0:1, :])
                ps_b = p_pmisc.tile([128, QS], _f32)
                for ch in range(2):
                    nc.tensor.matmul(ps_b[:, ts(ch, 512)], ones_r[0:1, :],
                                     s_row[:, ts(ch, 512)],
                                     start=True, stop=True)
                invS = p_keep.tile([128, QS], _f32, tag="invS")
                nc.vector.reciprocal(out=invS[:], in_=ps_b[:])

                # ---------------- normalize + store ----------------
                for i in range(NPT // 2):
                    st = p_stage.tile([128, 2, QS], _f32, tag="st")
                    for j in range(2):
                        nc.vector.tensor_mul(st[:, j, :],
                                             e_tiles[2 * i + j][:], invS[:])
                    nc.sync.dma_start(
                        out=corr_d[ds(256 * i, 256), :].rearrange(
                            "(j r) q -> r j q", j=2),
                        in_=st[:])

    nc.compile()
    return nc


# Q0 is a per-core column offset baked via partition id? No - inputs are
# sharded host-side, so the builder is shared: fa columns are pre-sliced on
# the host and DMA'd per core. Q0 below is only used to slice y (computed
# from the full fa on device). All cores run the same graph, so Q0 must be
# identical -> instead the host rotates fa_raw columns per core so that the
# local window is always [0, QS).
Q0 = 0


def _host_fc_path(fc_raw, Wc1, bc1, Wc2, bc2):
    def lrelu(x):
        return np.where(x >= 0, x, 0.2 * x).astype(np.float32)

    def inorm(x):
        m = x.mean(axis=(2, 3), keepdims=True)
        v = x.var(axis=(2, 3), keepdims=True)
        return ((x - m) / np.sqrt(v + EPS)).astype(np.float32)

    def conv(x, W, b, stride):
        n, ci, H, Wd = x.shape
        co = W.shape[0]
        Ho = (H - 3) // stride + 1
        Wo = (Wd - 3) // stride + 1
        out = np.zeros((n, co, Ho, Wo), np.float32)
        for dy in range(3):
            for dx in range(3):
                xs = x[:, :, dy:dy + stride * Ho:stride,
                       dx:dx + stride * Wo:stride]
                out += np.einsum("oi,niyx->noyx", W[:, :, dy, dx], xs,
                                 optimize=True)
        return out + b[None, :, None, None]

    x = np.pad(fc_raw, ((0, 0), (0, 0), (1, 1), (1, 1)), mode="reflect")
    x = lrelu(inorm(conv(x, Wc1, bc1, 2)))
    x = np.pad(x, ((0, 0), (0, 0), (1, 1), (1, 1)), mode="reflect")
    x = lrelu(inorm(conv(x, Wc2, bc2, 2)))
    return x  # [n, 3, 64, 64]


def _host_upsample(x, W, b):
    n, c, H, Wd = x.shape

    def coords(S):
        pos = np.arange(2 * S, dtype=np.float32) * (S - 1) / (2 * S - 1)
        i0 = np.floor(pos).astype(np.int32)
        i1 = np.minimum(i0 + 1, S - 1)
        return i0, i1, (pos - i0).astype(np.float32)

    i0, i1, fh = coords(H)
    x = x[:, :, i0, :] * (1 - fh)[None, None, :, None] + \
        x[:, :, i1, :] * fh[None, None, :, None]
    j0, j1, fw = coords(Wd)
    x = x[:, :, :, j0] * (1 - fw) + x[:, :, :, j1] * fw
    x = np.pad(x, ((0, 0), (0, 0), (1, 1), (1, 1)))
    # conv 3x3 stride 1 + IN + lrelu
    nH = x.shape[2] - 2
    nW = x.shape[3] - 2
    out = np.zeros((n, 3, nH, nW), np.float32)
    for dy in range(3):
        for dx in range(3):
            out += np.einsum("oi,niyx->noyx", W[:, :, dy, dx],
                             x[:, :, dy:dy + nH, dx:dx + nW], optimize=True)
    out += b[None, :, None, None]
    m = out.mean(axis=(2, 3), keepdims=True)
    v = out.var(axis=(2, 3), keepdims=True)
    out = (out - m) / np.sqrt(v + EPS)
    return np.where(out >= 0, out, 0.2 * out).astype(np.float32)


def kernel(fa_raw, fb_raw, fc_raw, Wa, ba, Wb, bb, Wc1, bc1, Wc2, bc2,
           Wu1, bu1, Wu2, bu2):
    fa_raw = np.asarray(fa_raw, np.float32)
    fb_raw = np.asarray(fb_raw, np.float32)
    fc_raw = np.asarray(fc_raw, np.float32)

    if "nc" not in _COMPILED:
        _COMPILED["nc"] = _build()
    nc = _COMPILED["nc"]

    fc = _host_fc_path(fc_raw, np.asarray(Wc1, np.float32),
                       np.asarray(bc1, np.float32),
                       np.asarray(Wc2, np.float32),
                       np.asarray(bc2, np.float32))
    fc_flat = fc.reshape(2, 3, HW)

    waT = np.ascontiguousarray(np.asarray(Wa, np.float32)[:, :, 0, 0].T)
    wbT = np.ascontiguousarray(np.asarray(Wb, np.float32)[:, :, 0, 0].T)

    in_maps = []
    for c in range(N_CORES):
        b, j = divmod(c, 4)
        q0 = j * QS
        # rotate fa columns so the local window is columns [0, QS)
        fa_b = fa_raw[b].reshape(C_IN, HW)
        fa_rot = np.concatenate([fa_b[:, q0:], fa_b[:, :q0]], axis=1)
        fct = np.zeros((128, NPT, 4), np.float32)
        fct[:, :, 0] = 1.0
        fct[:, :, 1:] = fc_flat[b].reshape(3, NPT, 128).transpose(2, 1, 0)
        in_maps.append({
            "fa_raw": np.ascontiguousarray(fa_rot),
            "fb_raw": np.ascontiguousarray(fb_raw[b].reshape(C_IN, HW)),
            "waT": waT,
            "wbT": wbT,
            "fct": fct.astype(ml_dtypes.bfloat16),
        })

    trace = bool(os.environ.get("BASS_TRACE"))
    res = run_bass_kernel_spmd(nc, in_maps, core_ids=list(range(N_CORES)),
                               trace=trace)
    LAST_RESULT["exec_time_ns"] = res.exec_time_ns

    corr = np.empty((2, HW, HW), np.float32)
    fcw = np.empty((2, 3, HW), np.float32)
    for c in range(N_CORES):
        b, j = divmod(c, 4)
        q0 = j * QS
        corr[b][:, q0:q0 + QS] = res.results[c]["corr"]
        numS = res.results[c]["numS"]
        fcw[b][:, q0:q0 + QS] = numS[1:4] / numS[0:1]

    x = fcw.reshape(2, 3, 64, 64)
    x = _host_upsample(x, np.asarray(Wu1, np.float32), np.asarray(bu1, np.float32))
    x = _host_upsample(x, np.asarray(Wu2, np.float32), np.asarray(bu2, np.float32))
    return x, corr
